# revision 34
# baseline (speedup 1.0000x reference)
"""Trainium2 Bass kernel for the cross-attention block nn_CA_54889682043704.

Reference computation (B=4, C=512, N=M=4096, da=128):
    q = w_qk @ x                      [B, da, N]
    k = w_qk @ y                      [B, da, M]
    v = w_v @ y + b_v                 [B, C, M]
    attn = softmax((q^T k) / sqrt(da), axis=M)
    x_s = v @ attn^T                  [B, C, N]
    out = relu(BN(w_t @ x_s + b_t)) transposed to [B, N, C]

Sharding: (batch b, query-half h) -> 8 cores, collective-free. Each core
computes the full attention for 2048 queries of one batch element.

fp8 (e4m3) DoubleRow pipeline: all projection/attention matmuls except the
energy run as fp8 DoubleRow pairs (K=256 per instruction, ~1.8x bf16 rate).
Weights are pre-scaled by 2^6 on the host (sigma=0.02 sits in e4m3's
subnormal range) and the scale is folded back out in the PSUM->SBUF casts.

Per-core dataflow:
    y, x arrive as fp8 chunk-major 3D tiles [128, 4, *].
    k_sb bf16 [da=128p, 4096] = (wk64^T y)*2^-6   (2 DR matmuls / 512-block)
    q_sb bf16 [da=128p, 2048] = (wk64^T x)*(2^-6/sqrt(da))
    vt pairs  [128p(m), 2, 512(c)] fp8 = (y^T wv64)*2^-6, per 128-key chunk
    per n-tile (512 queries), per key pair (2x128 keys):
      et [m128p, n512] = k-slice^T @ q-slice   (bf16 energy, 2 matmuls)
      pp[:, i, :] = exp(et_i)  fp8             (ACT, softmax shift skipped)
      S[c,n]    += vp-pair^T @ pp-pair         (4 DR, PSUM-accum over pairs)
      dn[1,n]   += ones-pair^T @ pp-pair       (1 DR: softmax denominator)
    tails (interleaved into the next tile's pair loop):
      dn -> bf16 (ACT, first in queue at the boundary); dt[n,1] = 16*dn via
      K=1 transpose matmuls; recip = 1/dt (so T*recip folds away the 2^6
      weight scale and the 1/4 S scale: 16 = 64/4)
      T[n128p, c512] = 16*S@W_eff   (2 DR matmuls per 128-query group)
      out = relu(T*recip + bias_eff)  (DVE scalar_tensor_tensor with exact
                                       f32 bias, then DVE max / ACT relu)

Scheduling notes (where the last ~50us came from):
  - the pair loop is software-pipelined by one pair so the in-order PE
    never waits on the ACT exp latency
  - input DMA is spread over the sync/gpsimd/scalar rings (~600ns issue
    cost each) in first-use order; weights are packed for 1-DMA loads
  - ~16 throwaway matmuls on memset data warm the PE clock ramp inside
    the initial DMA window
  - PSUM banks (8): 2 energy + 4 S-accum + 1 denominator + 1 tails

Host-side folding: b_v never reaches the device (softmax rows sum to 1 =>
w_t@b_v joins b_t); BN gamma/var folds into w_t (W_eff) and bias_eff.
"""

import sys

for _p in ("/opt/trn_rl_repo", "/root/.axon_site/_ro/trn_rl_repo"):
    if _p not in sys.path:
        sys.path.append(_p)

import math
import numpy as np
import ml_dtypes

import concourse.bacc as bacc
import concourse.bass as bass
import concourse.mybir as mybir
from concourse import tile
from concourse.bass_utils import run_bass_kernel_spmd

B, C, N, M = 4, 512, 4096, 4096
DA = 128
NCORES = 8
NL = N // 2            # queries per core
CCH = C // 128         # 4 channel chunks
MCH = M // 128         # 32 key chunks
NPAIR = MCH // 2       # 16 key pairs per n-tile
NTILES = NL // 512     # 4 query tiles per core
BN_EPS = 1e-5
SCALE = 1.0 / math.sqrt(DA)

WSH = 64.0             # host weight scale (2^6): lifts sigma=.02 out of
KS = 1.0 / WSH         # e4m3 subnormals; undone in the PSUM->SBUF casts
QS = SCALE / WSH
SSH = 4.0              # S stored as S/4 in fp8 (headroom below e4m3 max)
ECONST = WSH / SSH     # 16: combined scale recovered via recip = 1/(16*dn)

BF16 = mybir.dt.bfloat16
F32 = mybir.dt.float32
FP8 = mybir.dt.float8e4
NP_FP8 = ml_dtypes.float8_e4m3
NP_BF16 = ml_dtypes.bfloat16
PSUM = bass.MemorySpace.PSUM
DR = mybir.MatmulPerfMode.DoubleRow
EXP = mybir.ActivationFunctionType.Exp
COPY = mybir.ActivationFunctionType.Copy
MUL = mybir.AluOpType.mult
MAX = mybir.AluOpType.max


def build_program():
    nc = bacc.Bacc("TRN2", target_bir_lowering=False, debug=False,
                   num_devices=NCORES)

    # weights packed [128, CCH*F] so each loads as a single DMA; x/y stay
    # chunk-major so 512-column blocks can stream in first-use order
    xc_d = nc.dram_tensor("xc", [CCH, 128, NL], FP8, kind="ExternalInput").ap()
    yc_d = nc.dram_tensor("yc", [CCH, 128, M], FP8, kind="ExternalInput").ap()
    wk_d = nc.dram_tensor("wk", [128, CCH * DA], FP8,
                          kind="ExternalInput").ap()
    wv_d = nc.dram_tensor("wv", [128, CCH * C], FP8,
                          kind="ExternalInput").ap()
    wt_d = nc.dram_tensor("wt", [128, CCH * C], FP8,
                          kind="ExternalInput").ap()
    bb_d = nc.dram_tensor("bb", [128, C], F32, kind="ExternalInput").ap()
    out_d = nc.dram_tensor("out", [NL, C], F32, kind="ExternalOutput").ap()

    with tile.TileContext(nc) as tc:
        with (
            tc.tile_pool(name="persist", bufs=1) as wp,
            tc.tile_pool(name="vtp", bufs=NPAIR) as vtp,
            tc.tile_pool(name="ptp", bufs=6) as ptp,
            tc.tile_pool(name="ssb", bufs=2) as ssbp,
            tc.tile_pool(name="ep", bufs=4) as ep,
            tc.tile_pool(name="psA", bufs=2, space=PSUM) as psA,
            tc.tile_pool(name="psS", bufs=4, space=PSUM) as psS,
            tc.tile_pool(name="psD", bufs=1, space=PSUM) as psD,
            tc.tile_pool(name="psT", bufs=1, space=PSUM) as psT,
        ):
            ones8 = wp.tile([128, 2, 16], FP8, tag="ones8", name="ones8")
            nc.vector.memset(ones8[:], 1.0)
            c16 = wp.tile([1, 1], BF16, tag="c16", name="c16")
            nc.vector.memset(c16[:], ECONST)
            zb = wp.tile([128, 1], F32, tag="zb", name="zb")
            nc.vector.memset(zb[:], 0.0)

            # DMA issue costs ~600ns per dma_start on the issuing sequencer,
            # so the loads are spread over four rings ordered by first use:
            #   sync:   wk, then y chunks 0/1 (block 0 first)
            #   gpsimd: wv, then y chunks 2/3
            #   scalar: x (needed at the q projection, ~mid-prelude),
            #           then wt + bias row (needed at the first tails)
            wk3 = wp.tile([128, CCH, DA], FP8, tag="wk3", name="wk3")
            wv3 = wp.tile([128, CCH, C], FP8, tag="wv3", name="wv3")
            y3 = wp.tile([128, CCH, M], FP8, tag="y3", name="y3")
            nc.sync.dma_start(out=wk3[:], in_=wk_d)
            nc.gpsimd.dma_start(out=wv3[:], in_=wv_d)
            for mb in range(4):
                for ci in range(CCH):
                    eng = nc.sync if ci < 2 else nc.gpsimd
                    eng.dma_start(out=y3[:, ci, mb * 512:(mb + 1) * 512],
                                  in_=yc_d[ci, :, mb * 512:(mb + 1) * 512])
            x3 = wp.tile([128, CCH, NL], FP8, tag="x3", name="x3")
            for ci in range(CCH):
                nc.scalar.dma_start(out=x3[:, ci, :], in_=xc_d[ci])
            for ci in range(CCH):
                nc.scalar.dma_start(out=y3[:, ci, M // 2:],
                                    in_=yc_d[ci, :, M // 2:])
            wt3 = wp.tile([128, CCH, C], FP8, tag="wt3", name="wt3")
            nc.scalar.dma_start(out=wt3[:], in_=wt_d)
            bb = wp.tile([128, C], F32, tag="bb", name="bb")
            nc.scalar.dma_start(out=bb[:], in_=bb_d)

            q_sb = wp.tile([128, NL], BF16, tag="qsb", name="qsb")
            k_sb = wp.tile([128, M], BF16, tag="ksb", name="ksb")

            # PE pstate warmup: ~4us of throwaway matmuls on memset data
            # inside the initial DMA-wait window, so the clock ramp
            # (0.65 -> 2.4 GHz after ~3us busy) completes before real work
            warm = wp.tile([128, 640], BF16, tag="warm", name="warm")
            nc.vector.memset(warm[:], 0.5)
            for r in range(4):
                wps = psA.tile([128, 512], F32, tag="et", name=f"warm{r}")
                for rr in range(4):
                    nc.tensor.matmul(wps[:], lhsT=warm[:, 0:128],
                                     rhs=warm[:, 128:640],
                                     start=(rr == 0), stop=(rr == 3))

            # ---- k, vT (and q) per 512-key block, pipelined with the y DMA
            vt = [None] * NPAIR
            for mb in range(M // 512):
                ps = psA.tile([128, 512], F32, tag="et", name=f"kps{mb}")
                for g in range(2):
                    nc.tensor.matmul(
                        ps[:], lhsT=wk3[:, 2 * g:2 * g + 2, :],
                        rhs=y3[:, 2 * g:2 * g + 2, mb * 512:(mb + 1) * 512],
                        start=(g == 0), stop=(g == 1), perf_mode=DR)
                nc.vector.tensor_scalar(k_sb[:, mb * 512:(mb + 1) * 512],
                                        ps[:], KS, None, op0=MUL)
                for mj in range(mb * 4, mb * 4 + 4):
                    psv = psS.tile([128, C], F32, tag="s", name=f"vps{mj}")
                    for g in range(2):
                        nc.tensor.matmul(
                            psv[:],
                            lhsT=y3[:, 2 * g:2 * g + 2,
                                    mj * 128:(mj + 1) * 128],
                            rhs=wv3[:, 2 * g:2 * g + 2, :],
                            start=(g == 0), stop=(g == 1), perf_mode=DR)
                    j, i = divmod(mj, 2)
                    if i == 0:
                        vt[j] = vtp.tile([128, 2, C], FP8, tag="vt",
                                         name=f"vt{j}")
                    # alternate the evacuation between DVE and ACT so the
                    # prelude isn't cast-bound on a single engine
                    if mj % 2 == 0:
                        nc.vector.tensor_scalar(vt[j][:, i, :], psv[:],
                                                KS, None, op0=MUL)
                    else:
                        nc.scalar.activation(vt[j][:, i, :], psv[:], COPY,
                                             scale=KS)
                if mb == 1:
                    # q emitted here: PE fill work while y block 2 lands
                    for nt in range(NL // 512):
                        ps = psA.tile([128, 512], F32, tag="et",
                                      name=f"qps{nt}")
                        for g in range(2):
                            nc.tensor.matmul(
                                ps[:], lhsT=wk3[:, 2 * g:2 * g + 2, :],
                                rhs=x3[:, 2 * g:2 * g + 2,
                                       nt * 512:(nt + 1) * 512],
                                start=(g == 0), stop=(g == 1), perf_mode=DR)
                        nc.vector.tensor_scalar(
                            q_sb[:, nt * 512:(nt + 1) * 512], ps[:],
                            QS, None, op0=MUL)

            # ---- attention + output projection, one 512-query tile at a time
            # tail(0) converts the denominator; tail(1..4) emit one output
            # chunk each, spread across the next tile's pair loop so the
            # PE/ACT pipelines never drain at tile boundaries
            def make_tail(nt, dn16, s_sb):
                last = nt == NTILES - 1
                state = {}

                def tail0():
                    dt_ps = psT.tile([128, 4], F32, tag="t", name=f"dt{nt}")
                    for g in range(4):
                        nc.tensor.matmul(dt_ps[:, g:g + 1],
                                         lhsT=dn16[0:1, g * 128:(g + 1) * 128],
                                         rhs=c16[0:1, 0:1],
                                         start=True, stop=True)
                    recip = ep.tile([128, 4], F32, tag="recip",
                                    name=f"recip{nt}")
                    nc.vector.reciprocal(recip[:], dt_ps[:])
                    state["recip"] = recip

                def tail_g(g):
                    n0 = nt * 512
                    recip = state["recip"]
                    # final tile: psD is free after its dn16 copy, so
                    # alternate banks to unserialize the drain chain
                    pool = psD if (last and g % 2) else psT
                    t_ps = pool.tile([128, C], F32, tag="dn" if pool is psD
                                     else "t", name=f"t{nt}_{g}")
                    for gg in range(2):
                        nc.tensor.matmul(
                            t_ps[:],
                            lhsT=s_sb[:, 2 * gg:2 * gg + 2,
                                      g * 128:(g + 1) * 128],
                            rhs=wt3[:, 2 * gg:2 * gg + 2, :],
                            start=(gg == 0), stop=(gg == 1), perf_mode=DR)
                    u = ep.tile([128, C], F32, tag="o", name=f"u{nt}_{g}")
                    nc.vector.scalar_tensor_tensor(
                        u[:], t_ps[:], recip[:, g:g + 1], bb[:],
                        op0=MUL, op1=mybir.AluOpType.add)
                    o = ep.tile([128, C], F32, tag="o", name=f"o{nt}_{g}")
                    if last and g % 2:
                        # ACT only helps in the drain; mid-kernel it is the
                        # busier engine, so keep the relu on DVE there
                        nc.scalar.activation(
                            o[:], u[:], mybir.ActivationFunctionType.Relu)
                    else:
                        nc.vector.tensor_scalar_max(o[:], u[:], 0.0)
                    deng = nc.gpsimd if (last and g % 2) else nc.sync
                    deng.dma_start(
                        out=out_d[n0 + g * 128:n0 + (g + 1) * 128, :],
                        in_=o[:])

                return [tail0] + [lambda g=g: tail_g(g) for g in range(4)]

            pending_tails = []
            for nt in range(NTILES):
                n0 = nt * 512
                s_ps = [psS.tile([128, 512], F32, tag="s", name=f"s{nt}_{ci}")
                        for ci in range(CCH)]
                dn_ps = psD.tile([1, 512], F32, tag="dn", name=f"dn{nt}")

                def emit_sdn(pp, j):
                    def dn():
                        nc.tensor.matmul(dn_ps[:], lhsT=ones8[:, :, 0:1],
                                         rhs=pp[:, :, :],
                                         start=(j == 0),
                                         stop=(j == NPAIR - 1),
                                         perf_mode=DR)
                    if j == NPAIR - 1:
                        dn()        # last pair: free the denominator early
                    for ci in range(CCH):
                        nc.tensor.matmul(
                            s_ps[ci][:],
                            lhsT=vt[j][:, :, ci * 128:(ci + 1) * 128],
                            rhs=pp[:, :, :],
                            start=(j == 0), stop=(j == NPAIR - 1),
                            perf_mode=DR)
                    if j != NPAIR - 1:
                        dn()

                # software-pipelined by one pair: ets/exps of pair j are
                # emitted before the S/dn DR block of pair j-1, so the
                # in-order PE never stalls on the exp latency
                prev = None
                for j in range(NPAIR):
                    pp = ptp.tile([128, 2, 512], FP8, tag="pt",
                                  name=f"pt{nt}_{j}")
                    for i in range(2):
                        mj = 2 * j + i
                        et = psA.tile([128, 512], F32, tag="et",
                                      name=f"et{nt}_{mj}")
                        nc.tensor.matmul(et[:],
                                         lhsT=k_sb[:, mj * 128:(mj + 1) * 128],
                                         rhs=q_sb[:, n0:n0 + 512],
                                         start=True, stop=True)
                        nc.scalar.activation(pp[:, i, :], et[:], EXP,
                                             bias=zb[:])
                    if prev is not None:
                        emit_sdn(*prev)
                    prev = (pp, j)
                    if pending_tails and j in (0, 2, 5, 8, 11):
                        pending_tails.pop(0)()
                emit_sdn(*prev)

                # dn16 first in the ACT queue at the boundary, so the
                # next tile's dn-DR start and the dt transposes never wait
                dn16 = ep.tile([1, 512], BF16, tag="dn16", name=f"dn16_{nt}")
                nc.scalar.activation(dn16[:], dn_ps[:], COPY)
                # S -> SBUF fp8 pairs for use as the T-projection stationary;
                # split DVE/ACT so the tile-boundary handoff isn't serial
                s_sb = ssbp.tile([128, CCH, 512], FP8, tag="ssb",
                                 name=f"ssb{nt}")
                for ci in range(CCH):
                    # last boundary: ACT is idle after its dn16 copy, so
                    # split the casts to unserialize the drain
                    if nt == NTILES - 1 and ci % 2:
                        nc.scalar.activation(s_sb[:, ci, :], s_ps[ci][:],
                                             COPY, scale=1.0 / SSH)
                    else:
                        nc.vector.tensor_scalar(s_sb[:, ci, :], s_ps[ci][:],
                                                1.0 / SSH, None, op0=MUL)
                pending_tails = make_tail(nt, dn16, s_sb)
            for t in pending_tails:
                t()

    nc.compile()
    return nc


_PROG = None


def _get_prog():
    global _PROG
    if _PROG is None:
        _PROG = build_program()
    return _PROG


def _prep_in_maps(x, y, w_qk, w_v, b_v, w_t, b_t, gamma, beta, run_mean,
                  run_var):
    f32 = lambda a: np.asarray(a, dtype=np.float32)
    x, y = f32(x), f32(y)
    w_qk, w_v, b_v = f32(w_qk), f32(w_v), f32(b_v)
    w_t, b_t = f32(w_t), f32(b_t)
    gamma, beta = f32(gamma), f32(beta)
    run_mean, run_var = f32(run_mean), f32(run_var)

    inv = gamma / np.sqrt(run_var + BN_EPS)
    # b_v folded through attention (softmax rows sum to 1), BN folded into w_t
    b_t_eff = w_t @ b_v + b_t
    bias_eff = b_t_eff * inv + beta - run_mean * inv
    weffT = (w_t * inv[:, None]).T          # [c, o]

    def to8(a):
        return np.ascontiguousarray(a).astype(NP_FP8)

    def chunks3(a):                          # [C, F] -> [CCH, 128, F]
        return np.ascontiguousarray(a).reshape(CCH, 128, -1)

    def wpack(a):                            # [C, F] -> [128, CCH*F]
        f = a.shape[1]
        return np.ascontiguousarray(
            a.reshape(CCH, 128, f).transpose(1, 0, 2).reshape(128, CCH * f))

    wk_p = wpack(to8(w_qk.T * WSH))
    wv_p = wpack(to8(w_v.T * WSH))
    wt_p = wpack(to8(weffT * WSH))
    bb_h = np.ascontiguousarray(
        np.broadcast_to(bias_eff.astype(np.float32), (128, C)))

    y8 = [chunks3(to8(y[b])) for b in range(B)]
    x8 = [[chunks3(to8(x[b][:, h * NL:(h + 1) * NL])) for h in range(2)]
          for b in range(B)]

    in_maps = []
    for core in range(NCORES):
        b, h = divmod(core, 2)
        in_maps.append({
            "xc": x8[b][h], "yc": y8[b],
            "wk": wk_p, "wv": wv_p, "wt": wt_p, "bb": bb_h,
        })
    return in_maps


def run(trace=False, **inputs):
    nc = _get_prog()
    in_maps = _prep_in_maps(**inputs)
    res = run_bass_kernel_spmd(nc, in_maps, core_ids=list(range(NCORES)),
                               trace=trace)
    out = np.empty((B, N, C), np.float32)
    for core in range(NCORES):
        b, h = divmod(core, 2)
        out[b, h * NL:(h + 1) * NL, :] = res.results[core]["out"]
    return out, res


def kernel(**inputs):
    out, _ = run(trace=False, **inputs)
    return out


# revision 35
# speedup vs baseline: 1.0022x; 1.0022x over previous
"""Trainium2 Bass kernel for the cross-attention block nn_CA_54889682043704.

Reference computation (B=4, C=512, N=M=4096, da=128):
    q = w_qk @ x                      [B, da, N]
    k = w_qk @ y                      [B, da, M]
    v = w_v @ y + b_v                 [B, C, M]
    attn = softmax((q^T k) / sqrt(da), axis=M)
    x_s = v @ attn^T                  [B, C, N]
    out = relu(BN(w_t @ x_s + b_t)) transposed to [B, N, C]

Sharding: (batch b, query-half h) -> 8 cores, collective-free. Each core
computes the full attention for 2048 queries of one batch element.

fp8 (e4m3) DoubleRow pipeline: all projection/attention matmuls except the
energy run as fp8 DoubleRow pairs (K=256 per instruction, ~1.8x bf16 rate).
Weights are pre-scaled by 2^6 on the host (sigma=0.02 sits in e4m3's
subnormal range) and the scale is folded back out in the PSUM->SBUF casts.

Per-core dataflow:
    y, x arrive as fp8 chunk-major 3D tiles [128, 4, *].
    k_sb bf16 [da=128p, 4096] = (wk64^T y)*2^-6   (2 DR matmuls / 512-block)
    q_sb bf16 [da=128p, 2048] = (wk64^T x)*(2^-6/sqrt(da))
    vt pairs  [128p(m), 2, 512(c)] fp8 = (y^T wv64)*2^-6, per 128-key chunk
    per n-tile (512 queries), per key pair (2x128 keys):
      et [m128p, n512] = k-slice^T @ q-slice   (bf16 energy, 2 matmuls)
      pp[:, i, :] = exp(et_i)  fp8             (ACT, softmax shift skipped)
      S[c,n]    += vp-pair^T @ pp-pair         (4 DR, PSUM-accum over pairs)
      dn[1,n]   += ones-pair^T @ pp-pair       (1 DR: softmax denominator)
    tails (interleaved into the next tile's pair loop):
      dn -> bf16 (ACT, first in queue at the boundary); dt[n,1] = 16*dn via
      K=1 transpose matmuls; recip = 1/dt (so T*recip folds away the 2^6
      weight scale and the 1/4 S scale: 16 = 64/4)
      T[n128p, c512] = 16*S@W_eff   (2 DR matmuls per 128-query group)
      out = relu(T*recip + bias_eff)  (DVE scalar_tensor_tensor with exact
                                       f32 bias, then DVE max / ACT relu)

Scheduling notes (where the last ~50us came from):
  - the pair loop is software-pipelined by one pair so the in-order PE
    never waits on the ACT exp latency
  - input DMA is spread over the sync/gpsimd/scalar rings (~600ns issue
    cost each) in first-use order; weights are packed for 1-DMA loads
  - ~16 throwaway matmuls on memset data warm the PE clock ramp inside
    the initial DMA window
  - PSUM banks (8): 2 energy + 4 S-accum + 1 denominator + 1 tails

Host-side folding: b_v never reaches the device (softmax rows sum to 1 =>
w_t@b_v joins b_t); BN gamma/var folds into w_t (W_eff) and bias_eff.
"""

import sys

for _p in ("/opt/trn_rl_repo", "/root/.axon_site/_ro/trn_rl_repo"):
    if _p not in sys.path:
        sys.path.append(_p)

import math
import numpy as np
import ml_dtypes

import concourse.bacc as bacc
import concourse.bass as bass
import concourse.mybir as mybir
from concourse import tile
from concourse.bass_utils import run_bass_kernel_spmd

B, C, N, M = 4, 512, 4096, 4096
DA = 128
NCORES = 8
NL = N // 2            # queries per core
CCH = C // 128         # 4 channel chunks
MCH = M // 128         # 32 key chunks
NPAIR = MCH // 2       # 16 key pairs per n-tile
NTILES = NL // 512     # 4 query tiles per core
BN_EPS = 1e-5
SCALE = 1.0 / math.sqrt(DA)

WSH = 64.0             # host weight scale (2^6): lifts sigma=.02 out of
KS = 1.0 / WSH         # e4m3 subnormals; undone in the PSUM->SBUF casts
QS = SCALE / WSH
SSH = 4.0              # S stored as S/4 in fp8 (headroom below e4m3 max)
ECONST = WSH / SSH     # 16: combined scale recovered via recip = 1/(16*dn)

BF16 = mybir.dt.bfloat16
F32 = mybir.dt.float32
FP8 = mybir.dt.float8e4
NP_FP8 = ml_dtypes.float8_e4m3
NP_BF16 = ml_dtypes.bfloat16
PSUM = bass.MemorySpace.PSUM
DR = mybir.MatmulPerfMode.DoubleRow
EXP = mybir.ActivationFunctionType.Exp
COPY = mybir.ActivationFunctionType.Copy
MUL = mybir.AluOpType.mult
MAX = mybir.AluOpType.max


def build_program():
    nc = bacc.Bacc("TRN2", target_bir_lowering=False, debug=False,
                   num_devices=NCORES)

    # weights packed [128, CCH*F] so each loads as a single DMA; x/y stay
    # chunk-major so 512-column blocks can stream in first-use order
    xc_d = nc.dram_tensor("xc", [CCH, 128, NL], FP8, kind="ExternalInput").ap()
    yc_d = nc.dram_tensor("yc", [CCH, 128, M], FP8, kind="ExternalInput").ap()
    wk_d = nc.dram_tensor("wk", [128, CCH * DA], FP8,
                          kind="ExternalInput").ap()
    wv_d = nc.dram_tensor("wv", [128, CCH * C], FP8,
                          kind="ExternalInput").ap()
    wt_d = nc.dram_tensor("wt", [128, CCH * C], FP8,
                          kind="ExternalInput").ap()
    bb_d = nc.dram_tensor("bb", [128, C], F32, kind="ExternalInput").ap()
    out_d = nc.dram_tensor("out", [NL, C], F32, kind="ExternalOutput").ap()

    with tile.TileContext(nc) as tc:
        with (
            tc.tile_pool(name="persist", bufs=1) as wp,
            tc.tile_pool(name="vtp", bufs=NPAIR) as vtp,
            tc.tile_pool(name="ptp", bufs=5) as ptp,
            tc.tile_pool(name="ssb", bufs=2) as ssbp,
            tc.tile_pool(name="ep", bufs=4) as ep,
            tc.tile_pool(name="psA", bufs=2, space=PSUM) as psA,
            tc.tile_pool(name="psS", bufs=4, space=PSUM) as psS,
            tc.tile_pool(name="psD", bufs=1, space=PSUM) as psD,
            tc.tile_pool(name="psT", bufs=1, space=PSUM) as psT,
        ):
            ones8 = wp.tile([128, 2, 16], FP8, tag="ones8", name="ones8")
            nc.vector.memset(ones8[:], 1.0)
            c16 = wp.tile([1, 1], BF16, tag="c16", name="c16")
            nc.vector.memset(c16[:], ECONST)
            zb = wp.tile([128, 1], F32, tag="zb", name="zb")
            nc.vector.memset(zb[:], 0.0)

            # DMA issue costs ~600ns per dma_start on the issuing sequencer,
            # so the loads are spread over four rings ordered by first use:
            #   sync:   wk, then y chunks 0/1 (block 0 first)
            #   gpsimd: wv, then y chunks 2/3
            #   scalar: x (needed at the q projection, ~mid-prelude),
            #           then wt + bias row (needed at the first tails)
            wk3 = wp.tile([128, CCH, DA], FP8, tag="wk3", name="wk3")
            wv3 = wp.tile([128, CCH, C], FP8, tag="wv3", name="wv3")
            y3 = wp.tile([128, CCH, M], FP8, tag="y3", name="y3")
            nc.sync.dma_start(out=wk3[:], in_=wk_d)
            nc.gpsimd.dma_start(out=wv3[:], in_=wv_d)
            for mb in range(4):
                for ci in range(CCH):
                    eng = nc.sync if ci < 2 else nc.gpsimd
                    eng.dma_start(out=y3[:, ci, mb * 512:(mb + 1) * 512],
                                  in_=yc_d[ci, :, mb * 512:(mb + 1) * 512])
            x3 = wp.tile([128, CCH, NL], FP8, tag="x3", name="x3")
            for ci in range(CCH):
                nc.scalar.dma_start(out=x3[:, ci, :], in_=xc_d[ci])
            for ci in range(CCH):
                nc.scalar.dma_start(out=y3[:, ci, M // 2:],
                                    in_=yc_d[ci, :, M // 2:])
            wt3 = wp.tile([128, CCH, C], FP8, tag="wt3", name="wt3")
            nc.scalar.dma_start(out=wt3[:], in_=wt_d)
            bb = wp.tile([128, C], F32, tag="bb", name="bb")
            nc.scalar.dma_start(out=bb[:], in_=bb_d)

            q_sb = wp.tile([128, NL], BF16, tag="qsb", name="qsb")
            k_sb = wp.tile([128, M], BF16, tag="ksb", name="ksb")

            # PE pstate warmup: ~4us of throwaway matmuls on memset data
            # inside the initial DMA-wait window, so the clock ramp
            # (0.65 -> 2.4 GHz after ~3us busy) completes before real work
            warm = wp.tile([128, 640], BF16, tag="warm", name="warm")
            nc.vector.memset(warm[:], 0.5)
            for r in range(4):
                wps = psA.tile([128, 512], F32, tag="et", name=f"warm{r}")
                for rr in range(4):
                    nc.tensor.matmul(wps[:], lhsT=warm[:, 0:128],
                                     rhs=warm[:, 128:640],
                                     start=(rr == 0), stop=(rr == 3))

            # ---- k, vT (and q) per 512-key block, pipelined with the y DMA
            vt = [None] * NPAIR
            for mb in range(M // 512):
                ps = psA.tile([128, 512], F32, tag="et", name=f"kps{mb}")
                for g in range(2):
                    nc.tensor.matmul(
                        ps[:], lhsT=wk3[:, 2 * g:2 * g + 2, :],
                        rhs=y3[:, 2 * g:2 * g + 2, mb * 512:(mb + 1) * 512],
                        start=(g == 0), stop=(g == 1), perf_mode=DR)
                nc.vector.tensor_scalar(k_sb[:, mb * 512:(mb + 1) * 512],
                                        ps[:], KS, None, op0=MUL)
                for mj in range(mb * 4, mb * 4 + 4):
                    psv = psS.tile([128, C], F32, tag="s", name=f"vps{mj}")
                    for g in range(2):
                        nc.tensor.matmul(
                            psv[:],
                            lhsT=y3[:, 2 * g:2 * g + 2,
                                    mj * 128:(mj + 1) * 128],
                            rhs=wv3[:, 2 * g:2 * g + 2, :],
                            start=(g == 0), stop=(g == 1), perf_mode=DR)
                    j, i = divmod(mj, 2)
                    if i == 0:
                        vt[j] = vtp.tile([128, 2, C], FP8, tag="vt",
                                         name=f"vt{j}")
                    # alternate the evacuation between DVE and ACT so the
                    # prelude isn't cast-bound on a single engine
                    if mj % 2 == 0:
                        nc.vector.tensor_scalar(vt[j][:, i, :], psv[:],
                                                KS, None, op0=MUL)
                    else:
                        nc.scalar.activation(vt[j][:, i, :], psv[:], COPY,
                                             scale=KS)
                if mb == 1:
                    # q emitted here: PE fill work while y block 2 lands
                    for nt in range(NL // 512):
                        ps = psA.tile([128, 512], F32, tag="et",
                                      name=f"qps{nt}")
                        for g in range(2):
                            nc.tensor.matmul(
                                ps[:], lhsT=wk3[:, 2 * g:2 * g + 2, :],
                                rhs=x3[:, 2 * g:2 * g + 2,
                                       nt * 512:(nt + 1) * 512],
                                start=(g == 0), stop=(g == 1), perf_mode=DR)
                        nc.vector.tensor_scalar(
                            q_sb[:, nt * 512:(nt + 1) * 512], ps[:],
                            QS, None, op0=MUL)

            # ---- attention + output projection, one 512-query tile at a time
            # tail(0) converts the denominator; tail(1..4) emit one output
            # chunk each, spread across the next tile's pair loop so the
            # PE/ACT pipelines never drain at tile boundaries
            def make_tail(nt, dn16, s_sb):
                last = nt == NTILES - 1
                state = {}

                def tail0():
                    dt_ps = psT.tile([128, 4], F32, tag="t", name=f"dt{nt}")
                    for g in range(4):
                        nc.tensor.matmul(dt_ps[:, g:g + 1],
                                         lhsT=dn16[0:1, g * 128:(g + 1) * 128],
                                         rhs=c16[0:1, 0:1],
                                         start=True, stop=True)
                    recip = ep.tile([128, 4], F32, tag="recip",
                                    name=f"recip{nt}")
                    nc.vector.reciprocal(recip[:], dt_ps[:])
                    state["recip"] = recip

                def tail_g(g):
                    n0 = nt * 512
                    recip = state["recip"]
                    # final tile: psD is free after its dn16 copy, so
                    # alternate banks to unserialize the drain chain
                    pool = psD if (last and g % 2) else psT
                    t_ps = pool.tile([128, C], F32, tag="dn" if pool is psD
                                     else "t", name=f"t{nt}_{g}")
                    for gg in range(2):
                        nc.tensor.matmul(
                            t_ps[:],
                            lhsT=s_sb[:, 2 * gg:2 * gg + 2,
                                      g * 128:(g + 1) * 128],
                            rhs=wt3[:, 2 * gg:2 * gg + 2, :],
                            start=(gg == 0), stop=(gg == 1), perf_mode=DR)
                    u = ep.tile([128, C], F32, tag="o", name=f"u{nt}_{g}")
                    nc.vector.scalar_tensor_tensor(
                        u[:], t_ps[:], recip[:, g:g + 1], bb[:],
                        op0=MUL, op1=mybir.AluOpType.add)
                    o = ep.tile([128, C], F32, tag="o", name=f"o{nt}_{g}")
                    if last and g % 2:
                        # ACT only helps in the drain; mid-kernel it is the
                        # busier engine, so keep the relu on DVE there
                        nc.scalar.activation(
                            o[:], u[:], mybir.ActivationFunctionType.Relu)
                    else:
                        nc.vector.tensor_scalar_max(o[:], u[:], 0.0)
                    deng = nc.gpsimd if (last and g % 2) else nc.sync
                    deng.dma_start(
                        out=out_d[n0 + g * 128:n0 + (g + 1) * 128, :],
                        in_=o[:])

                return [tail0] + [lambda g=g: tail_g(g) for g in range(4)]

            pending_tails = []
            for nt in range(NTILES):
                n0 = nt * 512
                s_ps = [psS.tile([128, 512], F32, tag="s", name=f"s{nt}_{ci}")
                        for ci in range(CCH)]
                dn_ps = psD.tile([1, 512], F32, tag="dn", name=f"dn{nt}")

                def emit_sdn(pp, j):
                    def dn():
                        nc.tensor.matmul(dn_ps[:], lhsT=ones8[:, :, 0:1],
                                         rhs=pp[:, :, :],
                                         start=(j == 0),
                                         stop=(j == NPAIR - 1),
                                         perf_mode=DR)
                    if j == NPAIR - 1:
                        dn()        # last pair: free the denominator early
                    for ci in range(CCH):
                        nc.tensor.matmul(
                            s_ps[ci][:],
                            lhsT=vt[j][:, :, ci * 128:(ci + 1) * 128],
                            rhs=pp[:, :, :],
                            start=(j == 0), stop=(j == NPAIR - 1),
                            perf_mode=DR)
                    if j != NPAIR - 1:
                        dn()

                # software-pipelined by one pair: ets/exps of pair j are
                # emitted before the S/dn DR block of pair j-1, so the
                # in-order PE never stalls on the exp latency
                prev = None
                for j in range(NPAIR):
                    pp = ptp.tile([128, 2, 512], FP8, tag="pt",
                                  name=f"pt{nt}_{j}")
                    for i in range(2):
                        mj = 2 * j + i
                        et = psA.tile([128, 512], F32, tag="et",
                                      name=f"et{nt}_{mj}")
                        nc.tensor.matmul(et[:],
                                         lhsT=k_sb[:, mj * 128:(mj + 1) * 128],
                                         rhs=q_sb[:, n0:n0 + 512],
                                         start=True, stop=True)
                        nc.scalar.activation(pp[:, i, :], et[:], EXP,
                                             bias=zb[:])
                    if prev is not None:
                        emit_sdn(*prev)
                    prev = (pp, j)
                    if pending_tails and j in (0, 2, 5, 8, 11):
                        pending_tails.pop(0)()
                emit_sdn(*prev)

                # dn16 first in the ACT queue at the boundary, so the
                # next tile's dn-DR start and the dt transposes never wait
                dn16 = ep.tile([1, 512], BF16, tag="dn16", name=f"dn16_{nt}")
                nc.scalar.activation(dn16[:], dn_ps[:], COPY)
                # S -> SBUF fp8 pairs for use as the T-projection stationary;
                # split DVE/ACT so the tile-boundary handoff isn't serial
                s_sb = ssbp.tile([128, CCH, 512], FP8, tag="ssb",
                                 name=f"ssb{nt}")
                for ci in range(CCH):
                    # last boundary: ACT is idle after its dn16 copy, so
                    # split the casts to unserialize the drain
                    if nt == NTILES - 1 and ci % 2:
                        nc.scalar.activation(s_sb[:, ci, :], s_ps[ci][:],
                                             COPY, scale=1.0 / SSH)
                    else:
                        nc.vector.tensor_scalar(s_sb[:, ci, :], s_ps[ci][:],
                                                1.0 / SSH, None, op0=MUL)
                pending_tails = make_tail(nt, dn16, s_sb)
            for t in pending_tails:
                t()

    nc.compile()
    return nc


_PROG = None


def _get_prog():
    global _PROG
    if _PROG is None:
        _PROG = build_program()
    return _PROG


def _prep_in_maps(x, y, w_qk, w_v, b_v, w_t, b_t, gamma, beta, run_mean,
                  run_var):
    f32 = lambda a: np.asarray(a, dtype=np.float32)
    x, y = f32(x), f32(y)
    w_qk, w_v, b_v = f32(w_qk), f32(w_v), f32(b_v)
    w_t, b_t = f32(w_t), f32(b_t)
    gamma, beta = f32(gamma), f32(beta)
    run_mean, run_var = f32(run_mean), f32(run_var)

    inv = gamma / np.sqrt(run_var + BN_EPS)
    # b_v folded through attention (softmax rows sum to 1), BN folded into w_t
    b_t_eff = w_t @ b_v + b_t
    bias_eff = b_t_eff * inv + beta - run_mean * inv
    weffT = (w_t * inv[:, None]).T          # [c, o]

    def to8(a):
        return np.ascontiguousarray(a).astype(NP_FP8)

    def chunks3(a):                          # [C, F] -> [CCH, 128, F]
        return np.ascontiguousarray(a).reshape(CCH, 128, -1)

    def wpack(a):                            # [C, F] -> [128, CCH*F]
        f = a.shape[1]
        return np.ascontiguousarray(
            a.reshape(CCH, 128, f).transpose(1, 0, 2).reshape(128, CCH * f))

    wk_p = wpack(to8(w_qk.T * WSH))
    wv_p = wpack(to8(w_v.T * WSH))
    wt_p = wpack(to8(weffT * WSH))
    bb_h = np.ascontiguousarray(
        np.broadcast_to(bias_eff.astype(np.float32), (128, C)))

    y8 = [chunks3(to8(y[b])) for b in range(B)]
    x8 = [[chunks3(to8(x[b][:, h * NL:(h + 1) * NL])) for h in range(2)]
          for b in range(B)]

    in_maps = []
    for core in range(NCORES):
        b, h = divmod(core, 2)
        in_maps.append({
            "xc": x8[b][h], "yc": y8[b],
            "wk": wk_p, "wv": wv_p, "wt": wt_p, "bb": bb_h,
        })
    return in_maps


def run(trace=False, **inputs):
    nc = _get_prog()
    in_maps = _prep_in_maps(**inputs)
    res = run_bass_kernel_spmd(nc, in_maps, core_ids=list(range(NCORES)),
                               trace=trace)
    out = np.empty((B, N, C), np.float32)
    for core in range(NCORES):
        b, h = divmod(core, 2)
        out[b, h * NL:(h + 1) * NL, :] = res.results[core]["out"]
    return out, res


def kernel(**inputs):
    out, _ = run(trace=False, **inputs)
    return out
